# revision 39
# baseline (speedup 1.0000x reference)
"""Trainium2 Bass kernel for nn_AggregationMambaBlock.

Model: input x (4, 2048, 64) is split into two length-1024 halves (plus
time-reversed copies); four independent Mamba blocks (d_model=64,
d_inner=256, d_state=16, d_conv=4, dt_rank=4) process the four streams;
outputs are concatenated (time and feature axes) and passed through a
DyTanh (gamma * tanh(alpha*x + beta1) + beta).

Sharding: 8 cores = 4 blocks x 2 batch-halves. Zero cross-core
communication; the reversals / concats / transposes are host-side shard
glue. Each core computes its block's full Mamba on (2, 1024, 64) plus the
residual and the DyTanh for its 64-feature slice of the output.

Device algorithm highlights:
  - causal depthwise conv folded into the input projection: 4 accumulating
    PE matmuls with time-shifted views of the (64, T) input, using weights
    pre-scaled by the conv taps (computed on device).
  - the selective scan runs as 16 (states) x 2 (channel tiles) independent
    first-order recurrences via the DVE tensor_tensor_scan instruction
    (fp32 internal state; the scan is latency-bound at ~2.2 cyc/elem
    regardless of dtype), with exp(A_s * delta) produced on the scalar
    engine (ACT) using per-partition scale, and B/C time series broadcast
    across partitions by the GPSIMD partition_broadcast custom op
    (B/C rows are DMA-restaged to partition 0 first: compute engines
    require quad-aligned partition starts).
  - the elementwise dBx / C*h multiplies run in bf16 (DVE 2x mode); the
    sum over the 16 states runs on the otherwise-idle PE as identity
    matmuls accumulating in fp32 PSUM. The final rounding error is tiny
    (~1e-6) because the residual path and output projection dominate.
  - softplus is computed as ln(1+exp(x)) (no Softplus ACT table on this
    compiler), and the ACT function-table assignment is constrained so
    Exp/Ln share one table set (avoids 16 table reloads on the critical
    path).
"""

import os
import sys

os.environ.setdefault("MYCRO_LOCAL_CACHE", "1")
if "/opt/trn_rl_repo" not in sys.path:
    sys.path.insert(0, "/opt/trn_rl_repo")

import numpy as np
import ml_dtypes

import concourse.bass as bass
import concourse.bacc as bacc
import concourse.tile as tile
from concourse import library_config, mybir
from concourse.tile_rust import add_dep_helper

F32 = mybir.dt.float32
BF16 = mybir.dt.bfloat16
AL = mybir.AluOpType
AF = mybir.ActivationFunctionType

P = 128          # SBUF partitions
L = 1024         # per-sequence length
T = 2 * L        # tokens per core (2 sequences, concatenated on free dim)
DM = 64          # d_model
DI = 256         # d_inner
DS = 16          # d_state
DTR = 4          # dt_rank
DC = 4           # d_conv
NW = 512         # matmul N-tile width
NT = T // NW     # 4 N-tiles
XP = T + 2 * (DC - 1)  # padded x width: [pad3 | seq0 | pad3 | seq1]


def _rhs_off(nt: int, k: int) -> int:
    """Column in the padded x tile for token block nt, conv tap k.

    Tap k multiplies x[t - 3 + k]; column of token t of seq0 is 3 + t,
    of seq1 is (3 + L + 3) + (t - L). k=3 is the unshifted x."""
    if nt < NT // 2:
        return nt * NW + k
    return (L + DC - 1) + (nt - NT // 2) * NW + k


def _patched_act_tables(module_arch):
    """Exp and Ln both live in several ACT table sets; the assignment pass
    picks the first match, sending Exp to exp_and_others and Ln to
    natural_log, which ping-pongs table loads in the softplus. Restrict
    both to natural_log_exp_and_others (set ids keep matching
    act_info.json since only memberships are filtered, not order)."""
    import concourse.hw_specs as hw_specs
    t = hw_specs.get_activation_tables.__wrapped__(module_arch) if hasattr(
        hw_specs.get_activation_tables, "__wrapped__") else None
    if t is None:
        t = _ORIG_GET_ACT_TABLES(module_arch)
    EXP = AF.Exp
    LN = AF.Ln
    for name, funcs in t.items():
        if name != "natural_log_exp_and_others":
            funcs.discard(EXP)
            funcs.discard(LN)
    return t


_ORIG_GET_ACT_TABLES = None


def _build_program() -> bass.Bass:
    import concourse.hw_specs as hw_specs
    import concourse.bacc as bacc_mod
    global _ORIG_GET_ACT_TABLES
    _ORIG_GET_ACT_TABLES = hw_specs.get_activation_tables
    hw_specs.get_activation_tables = _patched_act_tables
    bacc_mod.get_activation_tables = _patched_act_tables
    try:
        return _build_program_inner()
    finally:
        hw_specs.get_activation_tables = _ORIG_GET_ACT_TABLES
        bacc_mod.get_activation_tables = _ORIG_GET_ACT_TABLES


def _build_program_inner() -> bass.Bass:
    nc = bacc.Bacc("TRN2")

    # ---- per-core inputs (host supplies layouts; see _make_in_map) ----
    d_xT = nc.dram_tensor("xT", [DM, T], F32, kind="ExternalInput")
    d_inwT = nc.dram_tensor("in_wT", [DM, 2 * DI], F32, kind="ExternalInput")
    d_convwT = nc.dram_tensor("conv_wT", [1, DC * DI], F32, kind="ExternalInput")
    d_convb = nc.dram_tensor("conv_b2", [P, 2], F32, kind="ExternalInput")
    d_xprojT = nc.dram_tensor("xproj_wT2", [P, 72], F32, kind="ExternalInput")
    d_dtwT = nc.dram_tensor("dt_wT", [DTR, DI], F32, kind="ExternalInput")
    d_dtb = nc.dram_tensor("dt_b2", [P, 2], F32, kind="ExternalInput")
    d_alog = nc.dram_tensor("A_log2", [P, 2 * DS], F32, kind="ExternalInput")
    d_dpar = nc.dram_tensor("D2", [P, 2], F32, kind="ExternalInput")
    d_outwT = nc.dram_tensor("out_wT2", [P, 2 * DM], F32, kind="ExternalInput")
    d_alpha = nc.dram_tensor("alpha_c", [DM, 1], F32, kind="ExternalInput")
    d_gamma = nc.dram_tensor("gamma_c", [DM, 1], F32, kind="ExternalInput")
    d_beta1 = nc.dram_tensor("beta1_c", [DM, 1], F32, kind="ExternalInput")
    d_beta = nc.dram_tensor("beta_c", [DM, 1], F32, kind="ExternalInput")
    d_ident = nc.dram_tensor("ident", [P, P], BF16, kind="ExternalInput")
    d_out = nc.dram_tensor("out64", [DM, T], F32, kind="ExternalOutput")

    with tile.TileContext(nc) as tc:
        import contextlib

        with contextlib.ExitStack() as ctx:
            consts = ctx.enter_context(tc.tile_pool(name="consts", bufs=1))
            big = ctx.enter_context(tc.tile_pool(name="big", bufs=1))
            scanp = ctx.enter_context(tc.tile_pool(name="scanp", bufs=2))
            outp = ctx.enter_context(tc.tile_pool(name="outp", bufs=1))
            psA = tc.alloc_tile_pool(name="psA", bufs=6, space="PSUM")
            psB = tc.alloc_tile_pool(name="psB", bufs=1, space="PSUM")

            # ---- load weights / constants ----
            def cload(name, dram, shape, dt=F32):
                t = consts.tile(shape, dt, tag=name, name=name)
                nc.sync.dma_start(out=t, in_=dram.ap())
                return t

            t_inwT = cload("in_wT", d_inwT, [DM, 2 * DI])
            t_convwT = cload("conv_wT", d_convwT, [1, DC * DI])
            t_convb = cload("conv_b2", d_convb, [P, 2])
            t_xprojT = cload("xproj_wT2", d_xprojT, [P, 72])
            t_dtwT = cload("dt_wT", d_dtwT, [DTR, DI])
            t_dtb = cload("dt_b2", d_dtb, [P, 2])
            t_alog = cload("A_log2", d_alog, [P, 2 * DS])
            t_dpar = cload("D2", d_dpar, [P, 2])
            t_outwT = cload("out_wT2", d_outwT, [P, 2 * DM])
            t_alpha = cload("alpha_c", d_alpha, [DM, 1])
            t_gamma = cload("gamma_c", d_gamma, [DM, 1])
            t_beta1 = cload("beta1_c", d_beta1, [DM, 1])
            t_beta = cload("beta_c", d_beta, [DM, 1])
            t_identbf = consts.tile([P, P], BF16, tag="ident", name="ident")
            nc.sync.dma_start(out=t_identbf, in_=d_ident.ap())

            # padded input x: [0:3]=0 | seq0 | [L+3:L+6]=0 | seq1
            t_xpad = big.tile([DM, XP], F32, tag="xpad", name="xpad")
            nc.vector.memset(t_xpad[:, 0:DC - 1], 0.0)
            nc.vector.memset(t_xpad[:, L + DC - 1:L + 2 * (DC - 1)], 0.0)
            nc.sync.dma_start(out=t_xpad[:, DC - 1:DC - 1 + L], in_=d_xT.ap()[:, 0:L])
            nc.sync.dma_start(out=t_xpad[:, L + 2 * (DC - 1):XP], in_=d_xT.ap()[:, L:T])

            # conv-scaled input projection weights:
            # cw[k][m, c] = in_wT[m, c] * conv_w[c, k]   (c in 0..255)
            t_cw = []
            for k in range(DC):
                bcw = consts.tile([DM, DI], F32, tag=f"bcw{k}", name=f"bcw{k}")
                nc.gpsimd.partition_broadcast(
                    bcw, t_convwT[0:1, k * DI:(k + 1) * DI])
                cwk = consts.tile([DM, DI], F32, tag=f"cw{k}", name=f"cw{k}")
                nc.vector.tensor_tensor(out=cwk, in0=t_inwT[:, 0:DI], in1=bcw,
                                        op=AL.mult)
                t_cw.append(cwk)

            silu_insts = []
            lnexp_insts = []
            # ---- stage A: in-proj + causal depthwise conv + SiLU; z + SiLU ----
            t_xin = [big.tile([P, T], F32, tag=f"xin{i}", name=f"xin{i}") for i in range(2)]
            t_zs = [big.tile([P, T], F32, tag=f"zs{i}", name=f"zs{i}") for i in range(2)]
            for nt in range(NT):
                for ft in range(2):
                    ps = psA.tile([P, NW], F32, tag="psA", name="psA")
                    for k in range(DC):
                        nc.tensor.matmul(
                            ps,
                            lhsT=t_cw[k][:, ft * P:(ft + 1) * P],
                            rhs=t_xpad[:, _rhs_off(nt, k):_rhs_off(nt, k) + NW],
                            start=(k == 0), stop=(k == DC - 1))
                    nc.scalar.activation(
                        out=t_xin[ft][:, nt * NW:(nt + 1) * NW], in_=ps,
                        func=AF.Silu, bias=t_convb[:, ft:ft + 1])
                for zf in range(2):
                    ps = psA.tile([P, NW], F32, tag="psA", name="psA")
                    nc.tensor.matmul(
                        ps,
                        lhsT=t_inwT[:, DI + zf * P:DI + (zf + 1) * P],
                        rhs=t_xpad[:, _rhs_off(nt, DC - 1):_rhs_off(nt, DC - 1) + NW],
                        start=True, stop=True)
                    zsi = nc.scalar.activation(
                        out=t_zs[zf][:, nt * NW:(nt + 1) * NW], in_=ps,
                        func=AF.Silu)
                    silu_insts.append(zsi)

            # negA[p, j*16+s] = -exp(A_log[j*128+p, s])
            t_expA = consts.tile([P, 2 * DS], F32, tag="expA", name="expA")
            lnexp_insts.append(
                nc.scalar.activation(out=t_expA, in_=t_alog, func=AF.Exp))
            t_negA = consts.tile([P, 2 * DS], F32, tag="negA", name="negA")
            nc.vector.tensor_scalar_mul(t_negA, t_expA, -1.0)

            # ---- stage B: x-proj (dt/B/C), delta = softplus(dt @ dt_w.T + b) ----
            t_xdbl = big.tile([DTR + 2 * DS, T], F32, tag="xdbl", name="xdbl")
            for nt in range(NT):
                ps36 = psB.tile([DTR + 2 * DS, NW], F32, tag="ps36", name="ps36")
                for kt in range(2):
                    nc.tensor.matmul(
                        ps36,
                        lhsT=t_xprojT[:, kt * 36:(kt + 1) * 36],
                        rhs=t_xin[kt][:, nt * NW:(nt + 1) * NW],
                        start=(kt == 0), stop=(kt == 1))
                nc.scalar.copy(out=t_xdbl[:, nt * NW:(nt + 1) * NW], in_=ps36)

            t_xdblbf = big.tile([DTR + 2 * DS, T], BF16, tag="xdblbf",
                                name="xdblbf")
            nc.scalar.copy(out=t_xdblbf, in_=t_xdbl)

            t_delta = [big.tile([P, T], F32, tag=f"delta{i}", name=f"delta{i}") for i in range(2)]
            t_u = [big.tile([P, T], BF16, tag=f"u{i}", name=f"u{i}") for i in range(2)]
            for di in range(2):
                for nt in range(NT):
                    psd = psB.tile([P, NW], F32, tag="psd", name="psd")
                    nc.tensor.matmul(
                        psd,
                        lhsT=t_dtwT[:, di * P:(di + 1) * P],
                        rhs=t_xdbl[0:DTR, nt * NW:(nt + 1) * NW],
                        start=True, stop=True)
                    # softplus(v + b) = ln(1 + exp(v + b)); Softplus has no
                    # ACT table set on this compiler, so exp then ln(1+x).
                    sptmp = scanp.tile([P, NW], F32, tag="sptmp", name="sptmp", bufs=2)
                    lnexp_insts.append(nc.scalar.activation(
                        out=sptmp, in_=psd,
                        func=AF.Exp, bias=t_dtb[:, di:di + 1]))
                    nc.scalar.activation(
                        out=t_delta[di][:, nt * NW:(nt + 1) * NW], in_=sptmp,
                        func=AF.Ln, bias=1.0)
                nc.vector.tensor_tensor(out=t_u[di], in0=t_delta[di],
                                        in1=t_xin[di], op=AL.mult)

            psB.release()
            psA.release()
            psY = tc.alloc_tile_pool(name="psY", bufs=1, space="PSUM")

            for le in lnexp_insts:
                for si in silu_insts:
                    add_dep_helper(le.ins, si.ins,
                                   reason="ACT table: silus before ln/exp")

            # ---- stage C: selective scan over 16 states ----
            # B/C rows are DMA-restaged to partition 0 (compute engines
            # require quad-aligned partition starts), then broadcast across
            # all 128 partitions by the GPSIMD partition_broadcast custom op
            # (the GPSIMD is otherwise idle). The sum over states runs on
            # the PE as identity-matmul accumulation into PSUM.
            t_y = [big.tile([P, T], F32, tag=f"y{i}", name=f"y{i}") for i in range(2)]
            y_ps = [psY.tile([P, T], F32, tag=f"yps{i}", name=f"yps{i}")
                    for i in range(2)]
            for s in range(DS):
                bsrc = scanp.tile([1, T], BF16, tag="rowstage", name="bsrc", bufs=2)
                nc.sync.dma_start(out=bsrc, in_=t_xdblbf[DTR + s:DTR + s + 1, :])
                csrc = scanp.tile([1, T], BF16, tag="rowstage", name="csrc", bufs=2)
                nc.sync.dma_start(out=csrc, in_=t_xdblbf[DTR + DS + s:DTR + DS + s + 1, :])
                bB = scanp.tile([P, T], BF16, tag="bB", name="bB", bufs=3)
                nc.gpsimd.partition_broadcast(bB, bsrc)
                bC = scanp.tile([P, T], BF16, tag="bC", name="bC", bufs=3)
                nc.gpsimd.partition_broadcast(bC, csrc)
                for di in range(2):
                    dA = scanp.tile([P, T], BF16, tag="dA", name="dA", bufs=4)
                    nc.scalar.activation(
                        out=dA, in_=t_delta[di], func=AF.Exp,
                        scale=t_negA[:, di * DS + s:di * DS + s + 1])
                    dbx = scanp.tile([P, T], BF16, tag="dbx", name="dbx", bufs=1)
                    nc.vector.tensor_tensor(out=dbx, in0=t_u[di], in1=bB,
                                            op=AL.mult)
                    h = scanp.tile([P, T], BF16, tag="h", name="h", bufs=1)
                    for q in range(2):
                        sl = slice(q * L, (q + 1) * L)
                        nc.vector.tensor_tensor_scan(
                            out=h[:, sl], data0=dA[:, sl], data1=dbx[:, sl],
                            initial=0.0, op0=AL.mult, op1=AL.add)
                    hh = scanp.tile([P, T], BF16, tag="hh", name="hh", bufs=4)
                    nc.vector.tensor_tensor(out=hh, in0=h, in1=bC,
                                            op=AL.mult)
                    for c in range(NT):
                        nc.tensor.matmul(
                            y_ps[di][:, c * NW:(c + 1) * NW],
                            lhsT=t_identbf,
                            rhs=hh[:, c * NW:(c + 1) * NW],
                            start=(s == 0), stop=(s == DS - 1))
            for di in range(2):
                for c in range(NT):
                    nc.scalar.copy(out=t_y[di][:, c * NW:(c + 1) * NW],
                                   in_=y_ps[di][:, c * NW:(c + 1) * NW])
            psY.release()
            psD = tc.alloc_tile_pool(name="psD", bufs=2, space="PSUM")

            # ---- stage D: +D*xin, gate by silu(z), out-proj, residual, DyTanh ----
            for di in range(2):
                nc.vector.scalar_tensor_tensor(
                    out=t_y[di], in0=t_xin[di], scalar=t_dpar[:, di:di + 1],
                    in1=t_y[di], op0=AL.mult, op1=AL.add)
                nc.vector.tensor_tensor(out=t_y[di], in0=t_y[di],
                                        in1=t_zs[di], op=AL.mult)
            for nt in range(NT):
                pso = psD.tile([DM, NW], F32, tag="pso", name="pso")
                for kt in range(2):
                    nc.tensor.matmul(
                        pso,
                        lhsT=t_outwT[:, kt * DM:(kt + 1) * DM],
                        rhs=t_y[kt][:, nt * NW:(nt + 1) * NW],
                        start=(kt == 0), stop=(kt == 1))
                pre = outp.tile([DM, NW], F32, tag="pre", name="pre")
                x0 = _rhs_off(nt, DC - 1)
                nc.vector.tensor_tensor(out=pre, in0=pso,
                                        in1=t_xpad[:, x0:x0 + NW], op=AL.add)
                th = outp.tile([DM, NW], F32, tag="th", name="th")
                nc.scalar.activation(out=th, in_=pre, func=AF.Tanh,
                                     scale=t_alpha[:, 0:1],
                                     bias=t_beta1[:, 0:1])
                ob = outp.tile([DM, NW], F32, tag="ob", name="ob")
                nc.vector.tensor_scalar(
                    out=ob, in0=th, scalar1=t_gamma[:, 0:1],
                    scalar2=t_beta[:, 0:1], op0=AL.mult, op1=AL.add)
                nc.sync.dma_start(out=d_out.ap()[:, nt * NW:(nt + 1) * NW], in_=ob)
            psD.release()

    nc.compile()
    return nc


_PROGRAM_CACHE: dict = {}


def _get_program() -> bass.Bass:
    if "nc" not in _PROGRAM_CACHE:
        _PROGRAM_CACHE["nc"] = _build_program()
    return _PROGRAM_CACHE["nc"]


def _make_in_maps(inputs: dict) -> list:
    """Build the 8 per-core input maps. Core (b, h) = blocks b in 0..3,
    batch half h in 0..1; core_id = b*2 + h."""
    x = np.asarray(inputs["x"], np.float32)          # (4, 2048, 64)
    in_w = np.asarray(inputs["in_w"], np.float32)    # (4, 512, 64)
    conv_w = np.asarray(inputs["conv_w"], np.float32)
    conv_b = np.asarray(inputs["conv_b"], np.float32)
    xproj_w = np.asarray(inputs["xproj_w"], np.float32)
    dt_w = np.asarray(inputs["dt_w"], np.float32)
    dt_b = np.asarray(inputs["dt_b"], np.float32)
    A_log = np.asarray(inputs["A_log"], np.float32)
    D_param = np.asarray(inputs["D_param"], np.float32)
    out_w = np.asarray(inputs["out_w"], np.float32)
    dy_alpha = np.asarray(inputs["dy_alpha"], np.float32).reshape(-1)[0]
    dy_beta = np.asarray(inputs["dy_beta"], np.float32).reshape(-1)
    dy_gamma = np.asarray(inputs["dy_gamma"], np.float32).reshape(-1)[0]
    dy_beta1 = np.asarray(inputs["dy_beta1"], np.float32).reshape(-1)

    x1 = x[:, :L]          # (4, 1024, 64)
    x2 = x[:, L:]
    streams = {0: x1[:, ::-1], 1: x2, 2: x1, 3: x2[:, ::-1]}

    in_maps = []
    for b in range(4):
        for h in range(2):
            t = streams[b][2 * h:2 * h + 2]           # (2, 1024, 64)
            xT = np.ascontiguousarray(
                t.reshape(T, DM).T)                   # (64, 2048)
            fh = slice(0, DM) if b < 2 else slice(DM, 2 * DM)
            m = {
                "xT": xT,
                "in_wT": np.ascontiguousarray(in_w[b].T),
                "conv_wT": np.ascontiguousarray(conv_w[b].T.reshape(1, DC * DI)),
                "conv_b2": np.ascontiguousarray(
                    conv_b[b].reshape(2, P).T),                    # (128, 2)
                "xproj_wT2": np.ascontiguousarray(
                    xproj_w[b].T.reshape(2, P, 36)
                    .transpose(1, 0, 2).reshape(P, 72)),
                "dt_wT": np.ascontiguousarray(dt_w[b].T),
                "dt_b2": np.ascontiguousarray(dt_b[b].reshape(2, P).T),
                "A_log2": np.ascontiguousarray(
                    A_log[b].reshape(2, P, DS)
                    .transpose(1, 0, 2).reshape(P, 2 * DS)),
                "D2": np.ascontiguousarray(D_param[b].reshape(2, P).T),
                "out_wT2": np.ascontiguousarray(
                    out_w[b].T.reshape(2, P, DM)
                    .transpose(1, 0, 2).reshape(P, 2 * DM)),
                "alpha_c": np.full((DM, 1), dy_alpha, np.float32),
                "gamma_c": np.full((DM, 1), dy_gamma, np.float32),
                "beta1_c": np.ascontiguousarray(
                    dy_beta1[fh].reshape(DM, 1)),
                "beta_c": np.ascontiguousarray(dy_beta[fh].reshape(DM, 1)),
                "ident": np.eye(P).astype(ml_dtypes.bfloat16),
            }
            in_maps.append(m)
    return in_maps


def _assemble(results: list) -> np.ndarray:
    """results[core]["out64"] (64, 2048) -> full (4, 2048, 128) output."""
    out = np.empty((4, T, 2 * DM), np.float32)
    for b in range(4):
        for h in range(2):
            o = results[b * 2 + h]["out64"]           # (64, 2048)
            ot = np.ascontiguousarray(o.T).reshape(2, L, DM)
            bs = slice(2 * h, 2 * h + 2)
            if b == 0:
                out[bs, 0:L, 0:DM] = ot[:, ::-1]
            elif b == 1:
                out[bs, L:T, 0:DM] = ot
            elif b == 2:
                out[bs, 0:L, DM:2 * DM] = ot
            else:
                out[bs, L:T, DM:2 * DM] = ot[:, ::-1]
    return out


def _exec(inputs: dict, trace: bool = False):
    from concourse.bass_utils import run_bass_kernel_spmd

    nc = _get_program()
    in_maps = _make_in_maps(inputs)
    r = run_bass_kernel_spmd(nc, in_maps, core_ids=list(range(8)), trace=trace)
    out = _assemble(r.results)
    return out, r


def kernel(**inputs) -> np.ndarray:
    out, _ = _exec(inputs, trace=False)
    return out


# revision 45
# speedup vs baseline: 1.0262x; 1.0262x over previous
"""Trainium2 Bass kernel for nn_AggregationMambaBlock.

Model: input x (4, 2048, 64) is split into two length-1024 halves (plus
time-reversed copies); four independent Mamba blocks (d_model=64,
d_inner=256, d_state=16, d_conv=4, dt_rank=4) process the four streams;
outputs are concatenated (time and feature axes) and passed through a
DyTanh (gamma * tanh(alpha*x + beta1) + beta).

Sharding: 8 cores = 4 blocks x 2 batch-halves. Zero cross-core
communication; the reversals / concats / transposes are host-side shard
glue. Each core computes its block's full Mamba on (2, 1024, 64) plus the
residual and the DyTanh for its 64-feature slice of the output.

Device algorithm highlights:
  - causal depthwise conv folded into the input projection: 4 accumulating
    PE matmuls with time-shifted views of the (64, T) input, using weights
    pre-scaled by the conv taps (computed on device).
  - the selective scan runs as 16 (states) x 2 (channel tiles) independent
    first-order recurrences via the DVE tensor_tensor_scan instruction
    (fp32 internal state; the scan is latency-bound at ~2.2 cyc/elem
    regardless of dtype), with exp(A_s * delta) produced on the scalar
    engine (ACT) using per-partition scale, and B/C time series broadcast
    across partitions by the GPSIMD partition_broadcast custom op
    (B/C rows are DMA-restaged to partition 0 first: compute engines
    require quad-aligned partition starts).
  - the elementwise dBx / C*h multiplies run in bf16 (DVE 2x mode); the
    sum over the 16 states runs on the otherwise-idle PE as identity
    matmuls accumulating in fp32 PSUM. The final rounding error is tiny
    (~1e-6) because the residual path and output projection dominate.
  - softplus is computed as ln(1+exp(x)) (no Softplus ACT table on this
    compiler), and the ACT function-table assignment is constrained so
    Exp/Ln share one table set (avoids 16 table reloads on the critical
    path).
"""

import os
import sys

os.environ.setdefault("MYCRO_LOCAL_CACHE", "1")
if "/opt/trn_rl_repo" not in sys.path:
    sys.path.insert(0, "/opt/trn_rl_repo")

import numpy as np
import ml_dtypes

import concourse.bass as bass
import concourse.bacc as bacc
import concourse.tile as tile
from concourse import library_config, mybir
from concourse.tile_rust import add_dep_helper

F32 = mybir.dt.float32
BF16 = mybir.dt.bfloat16
AL = mybir.AluOpType
AF = mybir.ActivationFunctionType

P = 128          # SBUF partitions
L = 1024         # per-sequence length
T = 2 * L        # tokens per core (2 sequences, concatenated on free dim)
DM = 64          # d_model
DI = 256         # d_inner
DS = 16          # d_state
DTR = 4          # dt_rank
DC = 4           # d_conv
NW = 512         # matmul N-tile width
NT = T // NW     # 4 N-tiles
XP = T + 2 * (DC - 1)  # padded x width: [pad3 | seq0 | pad3 | seq1]


def _rhs_off(nt: int, k: int) -> int:
    """Column in the padded x tile for token block nt, conv tap k.

    Tap k multiplies x[t - 3 + k]; column of token t of seq0 is 3 + t,
    of seq1 is (3 + L + 3) + (t - L). k=3 is the unshifted x."""
    if nt < NT // 2:
        return nt * NW + k
    return (L + DC - 1) + (nt - NT // 2) * NW + k


def _patched_act_tables(module_arch):
    """Exp and Ln both live in several ACT table sets; the assignment pass
    picks the first match, sending Exp to exp_and_others and Ln to
    natural_log, which ping-pongs table loads in the softplus. Restrict
    both to natural_log_exp_and_others (set ids keep matching
    act_info.json since only memberships are filtered, not order)."""
    import concourse.hw_specs as hw_specs
    t = hw_specs.get_activation_tables.__wrapped__(module_arch) if hasattr(
        hw_specs.get_activation_tables, "__wrapped__") else None
    if t is None:
        t = _ORIG_GET_ACT_TABLES(module_arch)
    EXP = AF.Exp
    LN = AF.Ln
    for name, funcs in t.items():
        if name != "natural_log_exp_and_others":
            funcs.discard(EXP)
            funcs.discard(LN)
    return t


_ORIG_GET_ACT_TABLES = None


def _build_program() -> bass.Bass:
    import concourse.hw_specs as hw_specs
    import concourse.bacc as bacc_mod
    global _ORIG_GET_ACT_TABLES
    _ORIG_GET_ACT_TABLES = hw_specs.get_activation_tables
    hw_specs.get_activation_tables = _patched_act_tables
    bacc_mod.get_activation_tables = _patched_act_tables
    try:
        return _build_program_inner()
    finally:
        hw_specs.get_activation_tables = _ORIG_GET_ACT_TABLES
        bacc_mod.get_activation_tables = _ORIG_GET_ACT_TABLES


def _build_program_inner() -> bass.Bass:
    nc = bacc.Bacc("TRN2")

    # ---- per-core inputs (host supplies layouts; see _make_in_map) ----
    d_xT = nc.dram_tensor("xT", [DM, T], F32, kind="ExternalInput")
    d_inwT = nc.dram_tensor("in_wT", [DM, 2 * DI], F32, kind="ExternalInput")
    d_convwT = nc.dram_tensor("conv_wT", [1, DC * DI], F32, kind="ExternalInput")
    d_convb = nc.dram_tensor("conv_b2", [P, 2], F32, kind="ExternalInput")
    d_xprojT = nc.dram_tensor("xproj_wT2", [P, 72], F32, kind="ExternalInput")
    d_dtwT = nc.dram_tensor("dt_wT", [DTR, DI], F32, kind="ExternalInput")
    d_dtb = nc.dram_tensor("dt_b2", [P, 2], F32, kind="ExternalInput")
    d_alog = nc.dram_tensor("A_log2", [P, 2 * DS], F32, kind="ExternalInput")
    d_dpar = nc.dram_tensor("D2", [P, 2], F32, kind="ExternalInput")
    d_outwT = nc.dram_tensor("out_wT2", [P, 2 * DM], F32, kind="ExternalInput")
    d_alpha = nc.dram_tensor("alpha_c", [DM, 1], F32, kind="ExternalInput")
    d_gamma = nc.dram_tensor("gamma_c", [DM, 1], F32, kind="ExternalInput")
    d_beta1 = nc.dram_tensor("beta1_c", [DM, 1], F32, kind="ExternalInput")
    d_beta = nc.dram_tensor("beta_c", [DM, 1], F32, kind="ExternalInput")
    d_ident = nc.dram_tensor("ident", [P, P], BF16, kind="ExternalInput")
    d_out = nc.dram_tensor("out64", [DM, T], F32, kind="ExternalOutput")

    with tile.TileContext(nc) as tc:
        import contextlib

        with contextlib.ExitStack() as ctx:
            consts = ctx.enter_context(tc.tile_pool(name="consts", bufs=1))
            big = ctx.enter_context(tc.tile_pool(name="big", bufs=1))
            scanp = ctx.enter_context(tc.tile_pool(name="scanp", bufs=2))
            outp = ctx.enter_context(tc.tile_pool(name="outp", bufs=1))
            psA = tc.alloc_tile_pool(name="psA", bufs=6, space="PSUM")
            psB = tc.alloc_tile_pool(name="psB", bufs=1, space="PSUM")

            # ---- load weights / constants ----
            def cload(name, dram, shape, dt=F32):
                t = consts.tile(shape, dt, tag=name, name=name)
                nc.sync.dma_start(out=t, in_=dram.ap())
                return t

            t_inwT = cload("in_wT", d_inwT, [DM, 2 * DI])
            t_convwT = cload("conv_wT", d_convwT, [1, DC * DI])
            t_convb = cload("conv_b2", d_convb, [P, 2])
            t_xprojT = cload("xproj_wT2", d_xprojT, [P, 72])
            t_dtwT = cload("dt_wT", d_dtwT, [DTR, DI])
            t_dtb = cload("dt_b2", d_dtb, [P, 2])
            t_alog = cload("A_log2", d_alog, [P, 2 * DS])
            t_dpar = cload("D2", d_dpar, [P, 2])
            t_outwT = cload("out_wT2", d_outwT, [P, 2 * DM])
            t_alpha = cload("alpha_c", d_alpha, [DM, 1])
            t_gamma = cload("gamma_c", d_gamma, [DM, 1])
            t_beta1 = cload("beta1_c", d_beta1, [DM, 1])
            t_beta = cload("beta_c", d_beta, [DM, 1])
            t_identbf = consts.tile([P, P], BF16, tag="ident", name="ident")
            nc.sync.dma_start(out=t_identbf, in_=d_ident.ap())

            # padded input x: [0:3]=0 | seq0 | [L+3:L+6]=0 | seq1
            t_xpad = big.tile([DM, XP], F32, tag="xpad", name="xpad")
            nc.vector.memset(t_xpad[:, 0:DC - 1], 0.0)
            nc.vector.memset(t_xpad[:, L + DC - 1:L + 2 * (DC - 1)], 0.0)
            nc.sync.dma_start(out=t_xpad[:, DC - 1:DC - 1 + L], in_=d_xT.ap()[:, 0:L])
            nc.sync.dma_start(out=t_xpad[:, L + 2 * (DC - 1):XP], in_=d_xT.ap()[:, L:T])

            # conv-scaled input projection weights:
            # cw[k][m, c] = in_wT[m, c] * conv_w[c, k]   (c in 0..255)
            t_cw = []
            for k in range(DC):
                bcw = consts.tile([DM, DI], F32, tag=f"bcw{k}", name=f"bcw{k}")
                nc.gpsimd.partition_broadcast(
                    bcw, t_convwT[0:1, k * DI:(k + 1) * DI])
                cwk = consts.tile([DM, DI], F32, tag=f"cw{k}", name=f"cw{k}")
                nc.vector.tensor_tensor(out=cwk, in0=t_inwT[:, 0:DI], in1=bcw,
                                        op=AL.mult)
                t_cw.append(cwk)

            silu_insts = []
            lnexp_insts = []
            # ---- stage A: in-proj + causal depthwise conv + SiLU; z + SiLU ----
            t_xin = [big.tile([P, T], F32, tag=f"xin{i}", name=f"xin{i}") for i in range(2)]
            t_zs = [big.tile([P, T], F32, tag=f"zs{i}", name=f"zs{i}") for i in range(2)]
            for nt in range(NT):
                for ft in range(2):
                    ps = psA.tile([P, NW], F32, tag="psA", name="psA")
                    for k in range(DC):
                        nc.tensor.matmul(
                            ps,
                            lhsT=t_cw[k][:, ft * P:(ft + 1) * P],
                            rhs=t_xpad[:, _rhs_off(nt, k):_rhs_off(nt, k) + NW],
                            start=(k == 0), stop=(k == DC - 1))
                    xsi = nc.scalar.activation(
                        out=t_xin[ft][:, nt * NW:(nt + 1) * NW], in_=ps,
                        func=AF.Silu, bias=t_convb[:, ft:ft + 1])
                    silu_insts.append(xsi)

            # negA[p, j*16+s] = -exp(A_log[j*128+p, s])
            t_expA = consts.tile([P, 2 * DS], F32, tag="expA", name="expA")
            lnexp_insts.append(
                nc.scalar.activation(out=t_expA, in_=t_alog, func=AF.Exp))
            t_negA = consts.tile([P, 2 * DS], F32, tag="negA", name="negA")
            nc.vector.tensor_scalar_mul(t_negA, t_expA, -1.0)

            # ---- stage B: x-proj (dt/B/C), delta = softplus(dt @ dt_w.T + b) ----
            t_xdbl = big.tile([DTR + 2 * DS, T], F32, tag="xdbl", name="xdbl")
            for nt in range(NT):
                ps36 = psB.tile([DTR + 2 * DS, NW], F32, tag="ps36", name="ps36")
                for kt in range(2):
                    nc.tensor.matmul(
                        ps36,
                        lhsT=t_xprojT[:, kt * 36:(kt + 1) * 36],
                        rhs=t_xin[kt][:, nt * NW:(nt + 1) * NW],
                        start=(kt == 0), stop=(kt == 1))
                nc.scalar.copy(out=t_xdbl[:, nt * NW:(nt + 1) * NW], in_=ps36)

            t_xdblbf = big.tile([DTR + 2 * DS, T], BF16, tag="xdblbf",
                                name="xdblbf")
            nc.scalar.copy(out=t_xdblbf, in_=t_xdbl)

            t_delta = [big.tile([P, T], F32, tag=f"delta{i}", name=f"delta{i}") for i in range(2)]
            t_u = [big.tile([P, T], BF16, tag=f"u{i}", name=f"u{i}") for i in range(2)]
            for di in range(2):
                for nt in range(NT):
                    psd = psB.tile([P, NW], F32, tag="psd", name="psd")
                    nc.tensor.matmul(
                        psd,
                        lhsT=t_dtwT[:, di * P:(di + 1) * P],
                        rhs=t_xdbl[0:DTR, nt * NW:(nt + 1) * NW],
                        start=True, stop=True)
                    # softplus(v + b) = ln(1 + exp(v + b)); Softplus has no
                    # ACT table set on this compiler, so exp then ln(1+x).
                    sptmp = scanp.tile([P, NW], F32, tag="sptmp", name="sptmp", bufs=2)
                    lnexp_insts.append(nc.scalar.activation(
                        out=sptmp, in_=psd,
                        func=AF.Exp, bias=t_dtb[:, di:di + 1]))
                    nc.scalar.activation(
                        out=t_delta[di][:, nt * NW:(nt + 1) * NW], in_=sptmp,
                        func=AF.Ln, bias=1.0)
                nc.vector.tensor_tensor(out=t_u[di], in0=t_delta[di],
                                        in1=t_xin[di], op=AL.mult)


            # z projection + SiLU: only needed at stage D, so emitted after
            # stage B to keep the PE off the pre-delta critical path.
            z_silus = []
            for nt in range(NT):
                for zf in range(2):
                    ps = psA.tile([P, NW], F32, tag="psA", name="psA")
                    nc.tensor.matmul(
                        ps,
                        lhsT=t_inwT[:, DI + zf * P:DI + (zf + 1) * P],
                        rhs=t_xpad[:, _rhs_off(nt, DC - 1):_rhs_off(nt, DC - 1) + NW],
                        start=True, stop=True)
                    zsi = nc.scalar.activation(
                        out=t_zs[zf][:, nt * NW:(nt + 1) * NW], in_=ps,
                        func=AF.Silu)
                    for le in lnexp_insts:
                        add_dep_helper(zsi.ins, le.ins,
                                       reason="ACT table: z-silus after ln/exp")
                    z_silus.append(zsi)

            psB.release()
            psA.release()
            psY = tc.alloc_tile_pool(name="psY", bufs=1, space="PSUM")

            for le in lnexp_insts:
                for si in silu_insts:
                    add_dep_helper(le.ins, si.ins,
                                   reason="ACT table: silus before ln/exp")

            # ---- stage C: selective scan over 16 states ----
            # B/C rows are DMA-restaged to partition 0 (compute engines
            # require quad-aligned partition starts), then broadcast across
            # all 128 partitions by the GPSIMD partition_broadcast custom op
            # (the GPSIMD is otherwise idle). The sum over states runs on
            # the PE as identity-matmul accumulation into PSUM.
            t_y = [big.tile([P, T], F32, tag=f"y{i}", name=f"y{i}") for i in range(2)]
            y_ps = [psY.tile([P, T], F32, tag=f"yps{i}", name=f"yps{i}")
                    for i in range(2)]
            for s in range(DS):
                bsrc = scanp.tile([1, T], BF16, tag="rowstage", name="bsrc", bufs=2)
                nc.sync.dma_start(out=bsrc, in_=t_xdblbf[DTR + s:DTR + s + 1, :])
                csrc = scanp.tile([1, T], BF16, tag="rowstage", name="csrc", bufs=2)
                nc.sync.dma_start(out=csrc, in_=t_xdblbf[DTR + DS + s:DTR + DS + s + 1, :])
                bB = scanp.tile([P, T], BF16, tag="bB", name="bB", bufs=3)
                nc.gpsimd.partition_broadcast(bB, bsrc)
                bC = scanp.tile([P, T], BF16, tag="bC", name="bC", bufs=3)
                nc.gpsimd.partition_broadcast(bC, csrc)
                for di in range(2):
                    dA = scanp.tile([P, T], BF16, tag="dA", name="dA", bufs=4)
                    dai = nc.scalar.activation(
                        out=dA, in_=t_delta[di], func=AF.Exp,
                        scale=t_negA[:, di * DS + s:di * DS + s + 1])
                    for zsi in z_silus:
                        add_dep_helper(dai.ins, zsi.ins,
                                       reason="ACT table: z-silus before dA")
                    dbx = scanp.tile([P, T], BF16, tag="dbx", name="dbx", bufs=1)
                    nc.vector.tensor_tensor(out=dbx, in0=t_u[di], in1=bB,
                                            op=AL.mult)
                    h = scanp.tile([P, T], BF16, tag="h", name="h", bufs=1)
                    for q in range(2):
                        sl = slice(q * L, (q + 1) * L)
                        nc.vector.tensor_tensor_scan(
                            out=h[:, sl], data0=dA[:, sl], data1=dbx[:, sl],
                            initial=0.0, op0=AL.mult, op1=AL.add)
                    hh = scanp.tile([P, T], BF16, tag="hh", name="hh", bufs=4)
                    nc.vector.tensor_tensor(out=hh, in0=h, in1=bC,
                                            op=AL.mult)
                    for c in range(NT):
                        nc.tensor.matmul(
                            y_ps[di][:, c * NW:(c + 1) * NW],
                            lhsT=t_identbf,
                            rhs=hh[:, c * NW:(c + 1) * NW],
                            start=(s == 0), stop=(s == DS - 1))
            for di in range(2):
                for c in range(NT):
                    nc.scalar.copy(out=t_y[di][:, c * NW:(c + 1) * NW],
                                   in_=y_ps[di][:, c * NW:(c + 1) * NW])
            psY.release()
            psD = tc.alloc_tile_pool(name="psD", bufs=2, space="PSUM")

            # ---- stage D: +D*xin, gate by silu(z), out-proj, residual, DyTanh ----
            for di in range(2):
                for nt in range(NT):
                    sl = slice(nt * NW, (nt + 1) * NW)
                    nc.vector.scalar_tensor_tensor(
                        out=t_y[di][:, sl], in0=t_xin[di][:, sl],
                        scalar=t_dpar[:, di:di + 1],
                        in1=t_y[di][:, sl], op0=AL.mult, op1=AL.add)
                    nc.vector.tensor_tensor(out=t_y[di][:, sl],
                                            in0=t_y[di][:, sl],
                                            in1=t_zs[di][:, sl], op=AL.mult)
            for nt in range(NT):
                pso = psD.tile([DM, NW], F32, tag="pso", name="pso")
                for kt in range(2):
                    nc.tensor.matmul(
                        pso,
                        lhsT=t_outwT[:, kt * DM:(kt + 1) * DM],
                        rhs=t_y[kt][:, nt * NW:(nt + 1) * NW],
                        start=(kt == 0), stop=(kt == 1))
                pre = outp.tile([DM, NW], F32, tag="pre", name="pre")
                x0 = _rhs_off(nt, DC - 1)
                nc.vector.tensor_tensor(out=pre, in0=pso,
                                        in1=t_xpad[:, x0:x0 + NW], op=AL.add)
                th = outp.tile([DM, NW], F32, tag="th", name="th")
                nc.scalar.activation(out=th, in_=pre, func=AF.Tanh,
                                     scale=t_alpha[:, 0:1],
                                     bias=t_beta1[:, 0:1])
                ob = outp.tile([DM, NW], F32, tag="ob", name="ob")
                nc.vector.tensor_scalar(
                    out=ob, in0=th, scalar1=t_gamma[:, 0:1],
                    scalar2=t_beta[:, 0:1], op0=AL.mult, op1=AL.add)
                nc.sync.dma_start(out=d_out.ap()[:, nt * NW:(nt + 1) * NW], in_=ob)
            psD.release()

    nc.compile()
    return nc


_PROGRAM_CACHE: dict = {}


def _get_program() -> bass.Bass:
    if "nc" not in _PROGRAM_CACHE:
        _PROGRAM_CACHE["nc"] = _build_program()
    return _PROGRAM_CACHE["nc"]


def _make_in_maps(inputs: dict) -> list:
    """Build the 8 per-core input maps. Core (b, h) = blocks b in 0..3,
    batch half h in 0..1; core_id = b*2 + h."""
    x = np.asarray(inputs["x"], np.float32)          # (4, 2048, 64)
    in_w = np.asarray(inputs["in_w"], np.float32)    # (4, 512, 64)
    conv_w = np.asarray(inputs["conv_w"], np.float32)
    conv_b = np.asarray(inputs["conv_b"], np.float32)
    xproj_w = np.asarray(inputs["xproj_w"], np.float32)
    dt_w = np.asarray(inputs["dt_w"], np.float32)
    dt_b = np.asarray(inputs["dt_b"], np.float32)
    A_log = np.asarray(inputs["A_log"], np.float32)
    D_param = np.asarray(inputs["D_param"], np.float32)
    out_w = np.asarray(inputs["out_w"], np.float32)
    dy_alpha = np.asarray(inputs["dy_alpha"], np.float32).reshape(-1)[0]
    dy_beta = np.asarray(inputs["dy_beta"], np.float32).reshape(-1)
    dy_gamma = np.asarray(inputs["dy_gamma"], np.float32).reshape(-1)[0]
    dy_beta1 = np.asarray(inputs["dy_beta1"], np.float32).reshape(-1)

    x1 = x[:, :L]          # (4, 1024, 64)
    x2 = x[:, L:]
    streams = {0: x1[:, ::-1], 1: x2, 2: x1, 3: x2[:, ::-1]}

    in_maps = []
    for b in range(4):
        for h in range(2):
            t = streams[b][2 * h:2 * h + 2]           # (2, 1024, 64)
            xT = np.ascontiguousarray(
                t.reshape(T, DM).T)                   # (64, 2048)
            fh = slice(0, DM) if b < 2 else slice(DM, 2 * DM)
            m = {
                "xT": xT,
                "in_wT": np.ascontiguousarray(in_w[b].T),
                "conv_wT": np.ascontiguousarray(conv_w[b].T.reshape(1, DC * DI)),
                "conv_b2": np.ascontiguousarray(
                    conv_b[b].reshape(2, P).T),                    # (128, 2)
                "xproj_wT2": np.ascontiguousarray(
                    xproj_w[b].T.reshape(2, P, 36)
                    .transpose(1, 0, 2).reshape(P, 72)),
                "dt_wT": np.ascontiguousarray(dt_w[b].T),
                "dt_b2": np.ascontiguousarray(dt_b[b].reshape(2, P).T),
                "A_log2": np.ascontiguousarray(
                    A_log[b].reshape(2, P, DS)
                    .transpose(1, 0, 2).reshape(P, 2 * DS)),
                "D2": np.ascontiguousarray(D_param[b].reshape(2, P).T),
                "out_wT2": np.ascontiguousarray(
                    out_w[b].T.reshape(2, P, DM)
                    .transpose(1, 0, 2).reshape(P, 2 * DM)),
                "alpha_c": np.full((DM, 1), dy_alpha, np.float32),
                "gamma_c": np.full((DM, 1), dy_gamma, np.float32),
                "beta1_c": np.ascontiguousarray(
                    dy_beta1[fh].reshape(DM, 1)),
                "beta_c": np.ascontiguousarray(dy_beta[fh].reshape(DM, 1)),
                "ident": np.eye(P).astype(ml_dtypes.bfloat16),
            }
            in_maps.append(m)
    return in_maps


def _assemble(results: list) -> np.ndarray:
    """results[core]["out64"] (64, 2048) -> full (4, 2048, 128) output."""
    out = np.empty((4, T, 2 * DM), np.float32)
    for b in range(4):
        for h in range(2):
            o = results[b * 2 + h]["out64"]           # (64, 2048)
            ot = np.ascontiguousarray(o.T).reshape(2, L, DM)
            bs = slice(2 * h, 2 * h + 2)
            if b == 0:
                out[bs, 0:L, 0:DM] = ot[:, ::-1]
            elif b == 1:
                out[bs, L:T, 0:DM] = ot
            elif b == 2:
                out[bs, 0:L, DM:2 * DM] = ot
            else:
                out[bs, L:T, DM:2 * DM] = ot[:, ::-1]
    return out


def _exec(inputs: dict, trace: bool = False):
    from concourse.bass_utils import run_bass_kernel_spmd

    nc = _get_program()
    in_maps = _make_in_maps(inputs)
    r = run_bass_kernel_spmd(nc, in_maps, core_ids=list(range(8)), trace=trace)
    out = _assemble(r.results)
    return out, r


def kernel(**inputs) -> np.ndarray:
    out, _ = _exec(inputs, trace=False)
    return out
